# revision 17
# baseline (speedup 1.0000x reference)
"""Trainium2 Bass kernel for nn_Attention_12429635355261 (sparse_attention).

Data-parallel over batch: 32 batch items -> 8 NeuronCores x 4.
Per core, batch items are processed in 2 pair-groups of 2 (free dims pack
the pair side-by-side so matmul moving free = 512).

v2 layout notes (vs v1):
  - DMA batching: conv gathers/scatters move whole multi-band groups with
    rearranged multi-dim APs (a dma_start costs ~700ns of engine time
    regardless of size, so count is what matters).
  - Spatial conv G tiles are column-padded ([128, nb, 2, 258]) so the three
    ki taps are plain offset slices -> exactly 3 matmuls per band.
  - Spectral branch: dots_s is produced directly in five overlapping
    126-row halo chunks (stationary col-windows of qsT), so the conv needs
    no halo gather at all; conv bands are 126 wide ([128,126] banded
    stationaries) and the final matmul contracts per-band.
  - attn@v uses tile_position col-tiling (m=64 pairs run concurrently).
  - Weights load from one blob dram tensor in a few large DMAs; output is
    bf16 (host casts back to fp32).
  - Pair-group emission interleaved (head1 between mid0 and tail0) to keep
    the PE dense so the HAM clock gate stays at 8/8.
"""
import os
import numpy as np

import concourse.bass as bass
import concourse.mybir as mybir
from concourse import bacc, tile
from concourse.bass_utils import run_bass_kernel_spmd

FP32 = mybir.dt.float32
BF16 = mybir.dt.bfloat16

H, D = 8, 64
T = 256
C = 512
SCALE = D ** -0.5
NCORES = 8
NB = 4            # batch per core
NPAIR = 2         # pair-groups per core

CONV_OUT = 14
NBANDS = 19
# G gather batches: (t0, t1, jt) ; jt None => band straddles the boundary
GBATCH = [(0, 9, 0), (9, 10, None), (10, 18, 1), (18, 19, 1)]

SB = 126
SCHUNKS = [(0, 126), (126, 126), (252, 126), (378, 126), (504, 8)]

_cache = {}


def _to_bf16(a):
    import ml_dtypes
    return np.asarray(a, np.float32).astype(ml_dtypes.bfloat16)


def _blob_layout():
    cols = [
        ("ident", 128),
        ("wspq", 2 * T),
        ("wspk", 2 * T),
        ("cstat", 3 * 112),
        ("sstat", 3 * SB),
        ("sstat0", 3 * SB),
        ("ones8", 64),
        ("sel16", 128),
        ("brow", 112),
        ("onesN", C),
        ("wout", 4 * C),
        ("wqkv", 4 * 3 * H * D),
    ]
    off = {}
    o = 0
    for name, n in cols:
        off[name] = (o, o + n)
        o += n
    return off, o


def build_conv_stationaries(w_sconv):
    LH = np.zeros((3, 128, 112), np.float32)
    for ki in range(3):
        for c in range(8):
            for rel in range(16):
                for o in range(8):
                    for jrel in range(14):
                        kj = rel - jrel
                        if 0 <= kj <= 2:
                            LH[ki, rel * 8 + c, jrel * 8 + o] = \
                                w_sconv[o, c, ki, kj]
    return LH


def build_spec_stationaries(w3, shift=0):
    # B[r, de, m] = w3[r - m + shift, de] for r - m + shift in {0,1,2}
    # shift=0: chunk row r maps to c = c0 - 1 + r (interior chunks)
    # shift=1: chunk row r maps to c = r (aligned chunk 0; c=-1 tap
    #          falls off the top of the stationary = implicit zero pad)
    B = np.zeros((128, 3, SB), np.float32)
    for r in range(128):
        for de in range(3):
            for m in range(SB):
                dc = r - m + shift
                if 0 <= dc <= 2:
                    B[r, de, m] = w3[dc, de]
    return B


def _ones8():
    s = np.zeros((128, 8, 8), np.float32)
    for h in range(8):
        s[:, h, h] = 1.0
    return s


def _sel16():
    s = np.zeros((8, 128), np.float32)
    for c in range(8):
        for rel in range(16):
            s[c, rel * 8 + c] = 1.0
    return s


def build_program(has_sconv_bias):
    nc = bacc.Bacc("TRN2", target_bir_lowering=False, debug=False)
    OFF, TOT = _blob_layout()

    x_d = nc.dram_tensor("x", [NB, T, C], BF16, kind="ExternalInput").ap()
    wb_d = nc.dram_tensor("wblob", [128, TOT], BF16, kind="ExternalInput").ap()
    fb_d = nc.dram_tensor("fblob", [128, 6], FP32, kind="ExternalInput").ap()
    out_d = nc.dram_tensor("out", [NB, T, C], BF16, kind="ExternalOutput").ap()

    from contextlib import ExitStack
    with ExitStack() as stk:
        tc = stk.enter_context(tile.TileContext(nc))
        pool = lambda name, bufs, **kw: stk.enter_context(
            tc.tile_pool(name=name, bufs=bufs, **kw))
        wres = pool("wres", 1)
        xin = pool("xin", 2)
        xtp = pool("xtp", 1)
        spqk = pool("spqk", 1)
        asp = pool("asp", 1)
        ascp = pool("ascp", 1)
        qkp = pool("qkp", 1)
        vp = pool("vp", 1)
        ep = pool("ep", 1)
        gp = pool("gp", 1)
        cgp = pool("cgp", 1)
        acp = pool("acp", 1)
        ofp = pool("ofp", 1)
        otp = pool("otp", 1)
        resp = pool("resp", 2)
        rp = pool("rp", 2)
        ps = pool("ps", 6, space="PSUM")
        pss = pool("pss", 1, space="PSUM")

        # ---------------- preload ----------------
        wb = wres.tile([128, TOT], BF16, tag="wb")
        fb = wres.tile([128, 6], FP32, tag="fb")

        def seg(*names):
            a = min(OFF[n][0] for n in names)
            b = max(OFF[n][1] for n in names)
            return a, b

        def ld(eng, a, b):
            eng.dma_start(wb[:, a:b], wb_d[:, a:b])

        x_t = {}

        def load_x(pg, bb, eng):
            t_ = xin.tile([128, 2, C], BF16, tag=f"x{bb}", name=f"x_{pg}_{bb}")
            eng.dma_start(t_[:], x_d[2 * pg + bb].rearrange(
                "(tt p) c -> p tt c", p=128))
            x_t.setdefault(pg, {})[bb] = t_

        qw = OFF["wqkv"][0]
        ld(nc.sync, *seg("ident"))
        load_x(0, 0, nc.sync)
        load_x(0, 1, nc.gpsimd)
        ld(nc.scalar, *seg("wspq", "wspk"))
        ld(nc.sync, qw, qw + 1536)
        ld(nc.scalar, qw + 1536, qw + 2 * 1536)
        ld(nc.gpsimd, qw + 2 * 1536, qw + 3 * 1536)
        ld(nc.sync, qw + 3 * 1536, qw + 4 * 1536)
        ld(nc.gpsimd, *seg("cstat", "sstat", "ones8", "sel16", "brow", "onesN"))
        ld(nc.scalar, *seg("wout"))
        nc.sync.dma_start(fb[:], fb_d[:])
        load_x(1, 0, nc.gpsimd)
        load_x(1, 1, nc.scalar)

        def V(name):
            a, b = OFF[name]
            return wb[:, a:b]

        ident = V("ident")
        wspq = V("wspq").rearrange("p (t x) -> p t x", t=2)
        wspk = V("wspk").rearrange("p (t x) -> p t x", t=2)
        cstat = V("cstat").rearrange("p (k x) -> p k x", k=3)
        sstat = V("sstat").rearrange("p (k x) -> p k x", k=3)
        sstat0 = V("sstat0").rearrange("p (k x) -> p k x", k=3)
        ones8 = V("ones8").rearrange("p (h x) -> p h x", h=8)
        sel16 = V("sel16")[0:8, :]
        brow = V("brow")[0:1, :]
        onesN = V("onesN")[0:1, :]
        wout = V("wout").rearrange("p (k x) -> p k x", k=4)
        wqkv = V("wqkv").rearrange("p (k x) -> p k x", k=4)
        b_out5 = fb[:, 0:5]
        bsp = fb[:, 5:6]

        EXP = mybir.ActivationFunctionType.Exp
        IDN = mybir.ActivationFunctionType.Identity
        MUL = mybir.AluOpType.mult
        ADD = mybir.AluOpType.add

        def copy_to(eng, dst, src):
            if eng is nc.scalar:
                eng.copy(dst, src)
            else:
                eng.tensor_copy(dst, src)

        st = {0: {}, 1: {}}

        # ================= head: transposes, spectral qk, dots_s =========
        def emit_head(pg):
            s = st[pg]
            xT2 = []
            for ct in range(4):
                dst = xtp.tile([128, 2 * T], BF16, tag=f"xT{ct}",
                               name=f"xT{ct}_{pg}")
                for bb in range(2):
                    pt = ps.tile([128, T], FP32, tag="pb", name="pt")
                    for tt in range(2):
                        nc.tensor.matmul(
                            pt[:, tt * 128:(tt + 1) * 128],
                            x_t[pg][bb][:, tt, ct * 128:(ct + 1) * 128],
                            ident, start=True, stop=True)
                    eng = nc.vector if (ct + bb) % 2 == 0 else nc.scalar
                    copy_to(eng, dst[:, bb * T:(bb + 1) * T], pt[:])
                xT2.append(dst)

            sq = [[None] * 2 for _ in range(2)]
            sk = [[None] * 2 for _ in range(2)]
            for bb in range(2):
                for ut in range(2):
                    for which, wsp, store in ((0, wspq, sq), (1, wspk, sk)):
                        pq = ps.tile([128, C], FP32, tag="pb", name="psq")
                        for tt in range(2):
                            nc.tensor.matmul(
                                pq[:], wsp[:, tt, ut * 128:(ut + 1) * 128],
                                x_t[pg][bb][:, tt, :],
                                start=(tt == 0), stop=(tt == 1))
                        t_ = spqk.tile([128, C], BF16, tag=f"sp{which}{bb}{ut}")
                        eng = nc.vector if (bb + ut + which) % 2 == 0 else nc.scalar
                        copy_to(eng, t_[:], pq[:])
                        store[bb][ut] = t_

            # dots_s in 5 overlapping halo chunks
            # chunk 0: row r <-> c = r (aligned; uses shifted stationary)
            # chunks 1-4: row r <-> c = c0 - 1 + r
            asb5 = {}
            for bb in range(2):
                a5 = asp.tile([128, 5, C], BF16, tag=f"as{bb}",
                              name=f"as{bb}_{pg}")
                ssum = rp.tile([128, 5], FP32, tag=f"ss{bb}")
                # zero the k=4 chunk's tail (row 9 is the conv pad row;
                # engine partition starts must be 32-aligned, so clear 0:32
                # first -- the exp overwrites rows 0..8 afterwards)
                nc.vector.memset(a5[0:32, 4, :], 0.0)
                for k, (c0, m) in enumerate(SCHUNKS):
                    if k == 0:
                        rhi, wlo, whi = 128, 0, 128
                    elif k == 4:
                        rhi, wlo, whi = 9, 503, 512
                    else:
                        rhi, wlo, whi = 128, c0 - 1, c0 + 127
                    pd = ps.tile([128, C], FP32, tag="pb", name="pds")
                    for ut in range(2):
                        nc.tensor.matmul(
                            pd[0:rhi, :], sq[bb][ut][:, wlo:whi],
                            sk[bb][ut][:], start=(ut == 0), stop=(ut == 1))
                    nc.scalar.activation(
                        a5[0:rhi, k, :], pd[0:rhi, :], EXP, scale=SCALE,
                        accum_out=ssum[0:rhi, k:k + 1])
                rr = rp.tile([128, 5], FP32, tag=f"rs{bb}")
                nc.vector.reciprocal(rr[:], ssum[:])
                for k in range(5):
                    rhi = 9 if k == 4 else 128
                    nc.gpsimd.tensor_scalar(a5[0:rhi, k, :], a5[0:rhi, k, :],
                                            rr[0:rhi, k:k + 1], None, MUL)
                asb5[bb] = a5
            s.update(xT2=xT2, sq=sq, sk=sk, asb5=asb5)

        # ================= mid: qkT, v, dots, denom, specconv, conv ======
        def emit_mid(pg):
            s = st[pg]
            xT2 = s["xT2"]

            qkT = []
            for mt in range(8):
                pq = ps.tile([128, 2 * T], FP32, tag="pb")
                for ct in range(4):
                    nc.tensor.matmul(
                        pq[:], wqkv[:, ct, mt * 128:(mt + 1) * 128],
                        xT2[ct][:], start=(ct == 0), stop=(ct == 3))
                t_ = qkp.tile([128, 2 * T], BF16, tag=f"qk{mt}")
                eng = nc.vector if mt % 2 == 0 else nc.scalar
                copy_to(eng, t_[:], pq[:])
                qkT.append(t_)

            v_sb = [[None] * 2 for _ in range(2)]
            for bb in range(2):
                for tt in range(2):
                    pv = ps.tile([128, C], FP32, tag="pb")
                    for ct in range(4):
                        nc.tensor.matmul(
                            pv[:],
                            xT2[ct][:, bb * T + tt * 128: bb * T + (tt + 1) * 128],
                            wqkv[:, ct, 2 * H * D:],
                            start=(ct == 0), stop=(ct == 3))
                    t_ = vp.tile([128, C], BF16, tag=f"v{bb}{tt}")
                    eng = nc.vector if (bb + tt) % 2 == 0 else nc.scalar
                    copy_to(eng, t_[:], pv[:])
                    v_sb[bb][tt] = t_

            # G batch tiles (one tile per jt-group of bands)
            gt = {}
            for bi, (t0, t1, jt) in enumerate(GBATCH):
                nb = t1 - t0
                g = gp.tile([128, nb, 2 * T], BF16, tag=f"g{bi}",
                            name=f"g{bi}_{pg}")
                gt[bi] = g
            nc.gpsimd.memset(gt[0][0:8, 0, :], 0.0)     # band0 rel0 pad
            # band18 tail pad: clear the whole band tile (full partition
            # range -- DVE partition windows are restricted); the gather
            # overwrites rows 0..40 afterwards
            nc.vector.memset(gt[3][:, 0, :], 0.0)

            E = ep.tile([128, H, 2, 2, T], BF16, tag="E", name=f"E_{pg}")

            def gather_batches(jt_done):
                # per-band single-dma gathers (the dma AP balancer caps
                # transfers at 3 refined dims, so bands can't be merged);
                # scalar is kept free of jt0-time issues (it is running exps)
                if jt_done == 0:
                    qs = (nc.sync, nc.gpsimd)
                    # band 0: rows -1..15 (rel 0 is the memset pad)
                    nc.sync.dma_start(gt[0][8:128, 0, :],
                                      E[0:15, :, 0, :, :])
                    for t in range(1, 9):
                        j0 = CONV_OUT * t
                        qs[t % 2].dma_start(gt[0][:, t, :],
                                            E[j0 - 1:j0 + 15, :, 0, :, :])
                    # band 9 jt0 part: rel 0..2 = rows 125..127
                    nc.gpsimd.dma_start(gt[1][0:24, 0, :],
                                        E[125:128, :, 0, :, :])
                else:
                    qs = (nc.sync, nc.scalar, nc.gpsimd)
                    # band 9 jt1 part: rel 3..15 = rows 128..140
                    nc.sync.dma_start(gt[1][24:128, 0, :],
                                      E[0:13, :, 1, :, :])
                    for t in range(10, 18):
                        j0 = CONV_OUT * t
                        qs[t % 3].dma_start(
                            gt[2][:, t - 10, :],
                            E[j0 - 129:j0 + 15 - 128, :, 1, :, :])
                    # band 18: rows 251..255 real, rest is the memset pad
                    nc.scalar.dma_start(gt[3][0:40, 0, :],
                                        E[123:128, :, 1, :, :])
            for jt in range(2):
                for h in range(H):
                    hp = 64 * (h % 2)
                    pd = ps.tile([128, 2 * T], FP32, tag="pb")
                    for bb in range(2):
                        nc.tensor.matmul(
                            pd[:, bb * T:(bb + 1) * T],
                            qkT[4 + h // 2][hp:hp + 64,
                                            bb * T + jt * 128: bb * T + (jt + 1) * 128],
                            qkT[h // 2][hp:hp + 64, bb * T:(bb + 1) * T],
                            start=True, stop=True)
                    nc.scalar.activation(E[:, h, jt, :, :], pd[:], EXP,
                                         scale=SCALE)
                gather_batches(jt)

            # softmax denominators
            sp16 = pss.tile([8, 2 * T], FP32, tag="s", name=f"sp_{pg}")
            for h in range(H):
                for jt in range(2):
                    nc.tensor.matmul(
                        sp16[:], ones8[:, h, :], E[:, h, jt, :, :],
                        start=(h == 0 and jt == 0),
                        stop=(h == H - 1 and jt == 1))
            rr16 = rp.tile([8, 2 * T], BF16, tag="rr")
            with nc.allow_low_precision(reason="bf16 recip"):
                nc.vector.reciprocal(rr16[:], sp16[:])

            # spectral conv (PE-filler while vector computes the recip)
            asc5 = {}
            for bb in range(2):
                a5 = s["asb5"][bb]
                c5 = ascp.tile([128, 5, C], BF16, tag=f"asc{bb}",
                               name=f"asc{bb}_{pg}")
                for k, (c0, m) in enumerate(SCHUNKS):
                    R = 128 if k < 4 else 10
                    ss = sstat0 if k == 0 else sstat
                    psc = ps.tile([128, C], FP32, tag="pb")
                    nc.tensor.matmul(psc[0:m, :], ss[0:R, 1, 0:m],
                                     a5[0:R, k, :], start=True, stop=False)
                    nc.tensor.matmul(psc[0:m, 1:C], ss[0:R, 0, 0:m],
                                     a5[0:R, k, 0:C - 1], start=False, stop=False)
                    nc.tensor.matmul(psc[0:m, 0:C - 1], ss[0:R, 2, 0:m],
                                     a5[0:R, k, 1:C], start=False, stop=True)
                    if (k + bb) % 2 == 0:
                        nc.scalar.activation(c5[0:m, k, :], psc[0:m, :], IDN,
                                             bias=bsp[0:m, 0:1])
                    else:
                        nc.vector.tensor_scalar(c5[0:m, k, :], psc[0:m, :],
                                                bsp[0:m, 0:1], None, ADD)
                asc5[bb] = c5

            Rg_ps = ps.tile([128, 2 * T], FP32, tag="pb", name="Rg_ps")
            nc.tensor.matmul(Rg_ps[:], sel16, rr16[:], start=True, stop=True)
            Rg = rp.tile([128, 2 * T], BF16, tag="Rg")
            nc.vector.tensor_copy(Rg[:], Rg_ps[:])
            Rg3 = Rg[:].rearrange("p (b i) -> p b i", b=2)

            # spatial conv
            AC = [acp.tile([128, H, 2, T], BF16, tag=f"AC{jt}",
                           name=f"AC{jt}_{pg}") for jt in range(2)]
            cgt = {bi: cgp.tile([112, t1 - t0, 2 * T], BF16, tag=f"cg{bi}",
                                name=f"cg{bi}_{pg}")
                   for bi, (t0, t1, jt) in enumerate(GBATCH)}

            for t in range(NBANDS):
                bi = next(i for i, (t0, t1, _) in enumerate(GBATCH)
                          if t0 <= t < t1)
                t0, t1, _ = GBATCH[bi]
                tb = t - t0
                g = gt[bi]
                nc.gpsimd.tensor_tensor(g[:, tb, :], g[:, tb, :], Rg[:], MUL)
                nwid = 112 if t < 18 else 32
                pc = ps.tile([112, 2 * T], FP32, tag="pb")
                nc.tensor.matmul(pc[0:nwid, :], cstat[:, 1, 0:nwid],
                                 g[:, tb, :], start=True, stop=False)
                for bb in range(2):
                    nc.tensor.matmul(
                        pc[0:nwid, bb * T + 1:(bb + 1) * T],
                        cstat[:, 0, 0:nwid],
                        g[:, tb, bb * T:(bb + 1) * T - 1],
                        start=False, stop=False)
                    nc.tensor.matmul(
                        pc[0:nwid, bb * T:(bb + 1) * T - 1],
                        cstat[:, 2, 0:nwid],
                        g[:, tb, bb * T + 1:(bb + 1) * T],
                        start=False,
                        stop=(bb == 1 and not has_sconv_bias))
                if has_sconv_bias:
                    nc.tensor.matmul(pc[0:nwid, :], brow[:, 0:nwid],
                                     onesN, start=False, stop=True)
                ceng = (nc.vector, nc.scalar)[t % 2]
                copy_to(ceng, cgt[bi][0:nwid, tb, :], pc[0:nwid, :])
                # per-band scatter into AC (one dma; band 9 straddles jt)
                cg = cgt[bi]
                sq3 = (nc.sync, nc.scalar)[t % 2]
                j0 = CONV_OUT * t
                if t < 9:
                    sq3.dma_start(AC[0][j0:j0 + 14], cg[:, tb, :])
                elif t == 9:
                    nc.sync.dma_start(AC[0][126:128], cg[0:16, tb, :])
                    nc.scalar.dma_start(AC[1][0:12], cg[16:112, tb, :])
                elif t < 18:
                    sq3.dma_start(AC[1][j0 - 128:j0 - 128 + 14],
                                  cg[:, tb, :])
                else:
                    sq3.dma_start(AC[1][124:128], cg[0:32, tb, :])

            s.update(qkT=qkT, v_sb=v_sb, AC=AC, asc5=asc5)

        # ================= tail: attn@v, proj, final, store ==============
        def emit_tail(pg):
            s = st[pg]
            v_sb, AC, asc5 = s["v_sb"], s["AC"], s["asc5"]

            OF = []
            for g4 in range(4):
                pav = ps.tile([128, 2 * T], FP32, tag="pb", name="pav")
                for hh in range(2):
                    h = 2 * g4 + hh
                    for bb in range(2):
                        for jt in range(2):
                            nc.tensor.matmul(
                                pav[64 * hh:64 * hh + 64, bb * T:(bb + 1) * T],
                                v_sb[bb][jt][:, h * D:(h + 1) * D],
                                AC[jt][:, h, bb, :],
                                start=(jt == 0), stop=(jt == 1),
                                tile_position=(0, 64 * hh))
                t_ = ofp.tile([128, 2 * T], BF16, tag=f"of{g4}")
                eng = nc.vector if g4 % 2 == 0 else nc.scalar
                copy_to(eng, t_[:], pav[:])
                OF.append(t_)

            outT5 = []
            for k, (c0, m) in enumerate(SCHUNKS):
                pp = ps.tile([128, 2 * T], FP32, tag="pb", name="pp")
                for kt in range(4):
                    nc.tensor.matmul(
                        pp[0:m, :], wout[:, kt, c0:c0 + m], OF[kt][:],
                        start=(kt == 0), stop=(kt == 3))
                t_ = otp.tile([128, 2 * T], BF16, tag=f"ot{k}")
                if k % 2 == 0:
                    nc.vector.tensor_scalar(t_[0:m, :], pp[0:m, :],
                                            b_out5[0:m, k:k + 1], None, ADD)
                else:
                    nc.scalar.activation(t_[0:m, :], pp[0:m, :], IDN,
                                         bias=b_out5[0:m, k:k + 1])
                outT5.append(t_)

            for bb in range(2):
                res = resp.tile([128, 2, C], BF16, tag=f"res{bb}")
                for tt2 in range(2):
                    pf = ps.tile([128, C], FP32, tag="pb", name="pf")
                    for k, (c0, m) in enumerate(SCHUNKS):
                        nc.tensor.matmul(
                            pf[:],
                            outT5[k][0:m, bb * T + tt2 * 128: bb * T + (tt2 + 1) * 128],
                            asc5[bb][0:m, k, :],
                            start=(k == 0), stop=(k == 4))
                    ceng = (nc.vector, nc.scalar)[(2 * bb + tt2) % 2]
                    copy_to(ceng, res[:, tt2, :], pf[:])
                (nc.sync if bb == 0 else nc.scalar).dma_start(
                    out_d[2 * pg + bb].rearrange("(tt p) c -> p tt c", p=128),
                    res[:])

        emit_head(0)
        emit_mid(0)
        emit_head(1)
        emit_tail(0)
        emit_mid(1)
        emit_tail(1)

    nc.compile()
    return nc


def _prep_inputs(inputs):
    x = np.asarray(inputs["x"], np.float32)
    w_qkv = np.asarray(inputs["w_qkv"], np.float32)
    w_out = np.asarray(inputs["w_out"], np.float32)
    b_out = np.asarray(inputs["b_out"], np.float32)
    w_sconv = np.asarray(inputs["w_sconv"], np.float32)
    b_sconv = np.asarray(inputs["b_sconv"], np.float32)
    w_specconv = np.asarray(inputs["w_specconv"], np.float32)
    b_specconv = np.asarray(inputs["b_specconv"], np.float32)
    w_qkv_spec = np.asarray(inputs["w_qkv_spec"], np.float32)

    has_sconv_bias = bool(np.any(b_sconv != 0))

    OFF, TOT = _blob_layout()
    blob = np.zeros((128, TOT), np.float32)

    def put(name, arr):
        a, b = OFF[name]
        blob[:arr.shape[0], a:b] = arr.reshape(arr.shape[0], b - a)

    put("ident", np.eye(128, dtype=np.float32))
    wspq = np.stack([w_qkv_spec[tt * 128:(tt + 1) * 128, :T]
                     for tt in range(2)], axis=1)
    put("wspq", wspq)
    wspk = np.stack([w_qkv_spec[tt * 128:(tt + 1) * 128, T:2 * T]
                     for tt in range(2)], axis=1)
    put("wspk", wspk)
    put("cstat", build_conv_stationaries(w_sconv).transpose(1, 0, 2))
    put("sstat", build_spec_stationaries(w_specconv[0, 0]))
    put("sstat0", build_spec_stationaries(w_specconv[0, 0], shift=1))
    put("ones8", _ones8())
    put("sel16", _sel16())
    put("brow", np.tile(b_sconv, CONV_OUT)[None, :])
    put("onesN", np.ones((1, C), np.float32))
    put("wout", np.stack([w_out[kt * 128:(kt + 1) * 128]
                          for kt in range(4)], axis=1))
    put("wqkv", np.stack([w_qkv[ct * 128:(ct + 1) * 128]
                          for ct in range(4)], axis=1))

    fblob = np.zeros((128, 6), np.float32)
    for k, (c0, m) in enumerate(SCHUNKS):
        fblob[0:m, k] = b_out[c0:c0 + m]
    fblob[:, 5] = b_specconv[0]

    common = {"wblob": _to_bf16(blob), "fblob": fblob}
    in_maps = []
    for core in range(NCORES):
        m = dict(common)
        m["x"] = _to_bf16(x[core * NB:(core + 1) * NB])
        in_maps.append(m)
    return in_maps, has_sconv_bias


def kernel(**inputs):
    in_maps, has_sconv_bias = _prep_inputs(inputs)
    key = ("v2", has_sconv_bias)
    if key not in _cache:
        _cache[key] = build_program(has_sconv_bias)
    nc = _cache[key]
    trace = bool(int(os.environ.get("KERNEL_TRACE", "0")))
    res = run_bass_kernel_spmd(nc, in_maps, list(range(NCORES)), trace=trace)
    if trace and res.exec_time_ns is not None:
        kernel.last_exec_time_ns = res.exec_time_ns
        kernel.last_profile = res
    out = np.concatenate(
        [np.asarray(res.results[i]["out"]).astype(np.float32)
         for i in range(NCORES)], axis=0)
    return out


kernel.last_exec_time_ns = None
kernel.last_profile = None


# revision 18
# speedup vs baseline: 1.5162x; 1.5162x over previous
"""Trainium2 Bass kernel for nn_Attention_12429635355261 (sparse_attention).

Data-parallel over batch: 32 batch items -> 8 NeuronCores x 4.
Per core, batch items are processed in 2 pair-groups of 2 (free dims pack
the pair side-by-side so matmul moving free = 512).

v2 layout notes (vs v1):
  - DMA batching: conv gathers/scatters move whole multi-band groups with
    rearranged multi-dim APs (a dma_start costs ~700ns of engine time
    regardless of size, so count is what matters).
  - Spatial conv G tiles are column-padded ([128, nb, 2, 258]) so the three
    ki taps are plain offset slices -> exactly 3 matmuls per band.
  - Spectral branch: dots_s is produced directly in five overlapping
    126-row halo chunks (stationary col-windows of qsT), so the conv needs
    no halo gather at all; conv bands are 126 wide ([128,126] banded
    stationaries) and the final matmul contracts per-band.
  - attn@v uses tile_position col-tiling (m=64 pairs run concurrently).
  - Weights load from one blob dram tensor in a few large DMAs; output is
    bf16 (host casts back to fp32).
  - Pair-group emission interleaved (head1 between mid0 and tail0) to keep
    the PE dense so the HAM clock gate stays at 8/8.
"""
import os
import numpy as np

import concourse.bass as bass
import concourse.mybir as mybir
from concourse import bacc, tile
from concourse.bass_utils import run_bass_kernel_spmd

FP32 = mybir.dt.float32
BF16 = mybir.dt.bfloat16

H, D = 8, 64
T = 256
C = 512
SCALE = D ** -0.5
NCORES = 8
NB = 4            # batch per core
NPAIR = 2         # pair-groups per core

CONV_OUT = 14
NBANDS = 19
# G gather batches: (t0, t1, jt) ; jt None => band straddles the boundary
GBATCH = [(0, 9, 0), (9, 10, None), (10, 18, 1), (18, 19, 1)]

SB = 126
SCHUNKS = [(0, 126), (126, 126), (252, 126), (378, 126), (504, 8)]

_cache = {}


def _to_bf16(a):
    import ml_dtypes
    return np.asarray(a, np.float32).astype(ml_dtypes.bfloat16)


def _blob_layout():
    cols = [
        ("ident", 128),
        ("wspq", 2 * T),
        ("wspk", 2 * T),
        ("cstat", 3 * 112),
        ("sstat", 3 * SB),
        ("sstat0", 3 * SB),
        ("ones8", 64),
        ("sel16", 128),
        ("brow", 112),
        ("onesN", C),
        ("wout", 4 * C),
        ("wqkv", 4 * 3 * H * D),
    ]
    off = {}
    o = 0
    for name, n in cols:
        off[name] = (o, o + n)
        o += n
    return off, o


def build_conv_stationaries(w_sconv):
    LH = np.zeros((3, 128, 112), np.float32)
    for ki in range(3):
        for c in range(8):
            for rel in range(16):
                for o in range(8):
                    for jrel in range(14):
                        kj = rel - jrel
                        if 0 <= kj <= 2:
                            LH[ki, rel * 8 + c, jrel * 8 + o] = \
                                w_sconv[o, c, ki, kj]
    return LH


def build_spec_stationaries(w3, shift=0):
    # B[r, de, m] = w3[r - m + shift, de] for r - m + shift in {0,1,2}
    # shift=0: chunk row r maps to c = c0 - 1 + r (interior chunks)
    # shift=1: chunk row r maps to c = r (aligned chunk 0; c=-1 tap
    #          falls off the top of the stationary = implicit zero pad)
    B = np.zeros((128, 3, SB), np.float32)
    for r in range(128):
        for de in range(3):
            for m in range(SB):
                dc = r - m + shift
                if 0 <= dc <= 2:
                    B[r, de, m] = w3[dc, de]
    return B


def _ones8():
    s = np.zeros((128, 8, 8), np.float32)
    for h in range(8):
        s[:, h, h] = 1.0
    return s


def _sel16():
    s = np.zeros((8, 128), np.float32)
    for c in range(8):
        for rel in range(16):
            s[c, rel * 8 + c] = 1.0
    return s


def build_program(has_sconv_bias):
    nc = bacc.Bacc("TRN2", target_bir_lowering=False, debug=False)
    OFF, TOT = _blob_layout()

    x_d = nc.dram_tensor("x", [NB, T, C], BF16, kind="ExternalInput").ap()
    wb_d = nc.dram_tensor("wblob", [128, TOT], BF16, kind="ExternalInput").ap()
    fb_d = nc.dram_tensor("fblob", [128, 6], FP32, kind="ExternalInput").ap()
    out_d = nc.dram_tensor("out", [NB, T, C], BF16, kind="ExternalOutput").ap()

    from contextlib import ExitStack
    with ExitStack() as stk:
        tc = stk.enter_context(tile.TileContext(nc))
        pool = lambda name, bufs, **kw: stk.enter_context(
            tc.tile_pool(name=name, bufs=bufs, **kw))
        wres = pool("wres", 1)
        xin = pool("xin", 2)
        xtp = pool("xtp", 1)
        spqk = pool("spqk", 1)
        asp = pool("asp", 1)
        ascp = pool("ascp", 1)
        qkp = pool("qkp", 1)
        vp = pool("vp", 1)
        ep = pool("ep", 1)
        gp = pool("gp", 1)
        cgp = pool("cgp", 1)
        acp = pool("acp", 1)
        ofp = pool("ofp", 1)
        otp = pool("otp", 1)
        resp = pool("resp", 2)
        rp = pool("rp", 2)
        ps = pool("ps", 6, space="PSUM")
        pss = pool("pss", 1, space="PSUM")

        # ---------------- preload ----------------
        wb = wres.tile([128, TOT], BF16, tag="wb")
        fb = wres.tile([128, 6], FP32, tag="fb")

        def seg(*names):
            a = min(OFF[n][0] for n in names)
            b = max(OFF[n][1] for n in names)
            return a, b

        def ld(eng, a, b):
            eng.dma_start(wb[:, a:b], wb_d[:, a:b])

        x_t = {}

        def load_x(pg, bb, eng):
            t_ = xin.tile([128, 2, C], BF16, tag=f"x{bb}", name=f"x_{pg}_{bb}")
            eng.dma_start(t_[:], x_d[2 * pg + bb].rearrange(
                "(tt p) c -> p tt c", p=128))
            x_t.setdefault(pg, {})[bb] = t_

        qw = OFF["wqkv"][0]
        ld(nc.sync, *seg("ident"))
        load_x(0, 0, nc.sync)
        load_x(0, 1, nc.gpsimd)
        ld(nc.scalar, *seg("wspq", "wspk"))
        ld(nc.sync, qw, qw + 1536)
        ld(nc.scalar, qw + 1536, qw + 2 * 1536)
        ld(nc.gpsimd, qw + 2 * 1536, qw + 3 * 1536)
        ld(nc.sync, qw + 3 * 1536, qw + 4 * 1536)
        ld(nc.gpsimd, *seg("cstat", "sstat", "ones8", "sel16", "brow", "onesN"))
        ld(nc.scalar, *seg("wout"))
        nc.sync.dma_start(fb[:], fb_d[:])
        load_x(1, 0, nc.gpsimd)
        load_x(1, 1, nc.scalar)

        def V(name):
            a, b = OFF[name]
            return wb[:, a:b]

        ident = V("ident")
        wspq = V("wspq").rearrange("p (t x) -> p t x", t=2)
        wspk = V("wspk").rearrange("p (t x) -> p t x", t=2)
        cstat = V("cstat").rearrange("p (k x) -> p k x", k=3)
        sstat = V("sstat").rearrange("p (k x) -> p k x", k=3)
        sstat0 = V("sstat0").rearrange("p (k x) -> p k x", k=3)
        ones8 = V("ones8").rearrange("p (h x) -> p h x", h=8)
        sel16 = V("sel16")[0:8, :]
        brow = V("brow")[0:1, :]
        onesN = V("onesN")[0:1, :]
        wout = V("wout").rearrange("p (k x) -> p k x", k=4)
        wqkv = V("wqkv").rearrange("p (k x) -> p k x", k=4)
        b_out5 = fb[:, 0:5]
        bsp = fb[:, 5:6]

        EXP = mybir.ActivationFunctionType.Exp
        IDN = mybir.ActivationFunctionType.Identity
        MUL = mybir.AluOpType.mult
        ADD = mybir.AluOpType.add

        def copy_to(eng, dst, src):
            if eng is nc.scalar:
                eng.copy(dst, src)
            else:
                eng.tensor_copy(dst, src)

        st = {0: {}, 1: {}}

        # ================= head: transposes, spectral qk, dots_s =========
        def emit_head(pg):
            s = st[pg]
            xT2 = []
            for ct in range(4):
                dst = xtp.tile([128, 2 * T], BF16, tag=f"xT{ct}",
                               name=f"xT{ct}_{pg}")
                for bb in range(2):
                    pt = ps.tile([128, T], FP32, tag="pb", name="pt")
                    for tt in range(2):
                        nc.tensor.matmul(
                            pt[:, tt * 128:(tt + 1) * 128],
                            x_t[pg][bb][:, tt, ct * 128:(ct + 1) * 128],
                            ident, start=True, stop=True)
                    eng = nc.vector if (ct + bb) % 2 == 0 else nc.scalar
                    copy_to(eng, dst[:, bb * T:(bb + 1) * T], pt[:])
                xT2.append(dst)

            sq = [[None] * 2 for _ in range(2)]
            sk = [[None] * 2 for _ in range(2)]
            for bb in range(2):
                for ut in range(2):
                    for which, wsp, store in ((0, wspq, sq), (1, wspk, sk)):
                        pq = ps.tile([128, C], FP32, tag="pb", name="psq")
                        for tt in range(2):
                            nc.tensor.matmul(
                                pq[:], wsp[:, tt, ut * 128:(ut + 1) * 128],
                                x_t[pg][bb][:, tt, :],
                                start=(tt == 0), stop=(tt == 1))
                        t_ = spqk.tile([128, C], BF16, tag=f"sp{which}{bb}{ut}")
                        eng = nc.vector if (bb + ut + which) % 2 == 0 else nc.scalar
                        copy_to(eng, t_[:], pq[:])
                        store[bb][ut] = t_

            # dots_s in 5 overlapping halo chunks
            # chunk 0: row r <-> c = r (aligned; uses shifted stationary)
            # chunks 1-4: row r <-> c = c0 - 1 + r
            asb5 = {}
            for bb in range(2):
                a5 = asp.tile([128, 5, C], BF16, tag=f"as{bb}",
                              name=f"as{bb}_{pg}")
                ssum = rp.tile([128, 5], FP32, tag=f"ss{bb}")
                # zero the k=4 chunk's tail (row 9 is the conv pad row;
                # engine partition starts must be 32-aligned, so clear 0:32
                # first -- the exp overwrites rows 0..8 afterwards)
                nc.vector.memset(a5[0:32, 4, :], 0.0)
                for k, (c0, m) in enumerate(SCHUNKS):
                    if k == 0:
                        rhi, wlo, whi = 128, 0, 128
                    elif k == 4:
                        rhi, wlo, whi = 9, 503, 512
                    else:
                        rhi, wlo, whi = 128, c0 - 1, c0 + 127
                    pd = ps.tile([128, C], FP32, tag="pb", name="pds")
                    for ut in range(2):
                        nc.tensor.matmul(
                            pd[0:rhi, :], sq[bb][ut][:, wlo:whi],
                            sk[bb][ut][:], start=(ut == 0), stop=(ut == 1))
                    nc.scalar.activation(
                        a5[0:rhi, k, :], pd[0:rhi, :], EXP, scale=SCALE,
                        accum_out=ssum[0:rhi, k:k + 1])
                rr = rp.tile([128, 5], FP32, tag=f"rs{bb}")
                nc.vector.reciprocal(rr[:], ssum[:])
                for k in range(5):
                    rhi = 9 if k == 4 else 128
                    if k % 2 == 0:
                        nc.scalar.activation(a5[0:rhi, k, :], a5[0:rhi, k, :],
                                             IDN, scale=rr[0:rhi, k:k + 1])
                    else:
                        nc.vector.tensor_scalar(a5[0:rhi, k, :],
                                                a5[0:rhi, k, :],
                                                rr[0:rhi, k:k + 1], None, MUL)
                asb5[bb] = a5
            s.update(xT2=xT2, sq=sq, sk=sk, asb5=asb5)

        # ================= mid: qkT, v, dots, denom, specconv, conv ======
        def emit_mid(pg):
            s = st[pg]
            xT2 = s["xT2"]

            qkT = []
            for mt in range(8):
                pq = ps.tile([128, 2 * T], FP32, tag="pb")
                for ct in range(4):
                    nc.tensor.matmul(
                        pq[:], wqkv[:, ct, mt * 128:(mt + 1) * 128],
                        xT2[ct][:], start=(ct == 0), stop=(ct == 3))
                t_ = qkp.tile([128, 2 * T], BF16, tag=f"qk{mt}")
                eng = nc.vector if mt % 2 == 0 else nc.scalar
                copy_to(eng, t_[:], pq[:])
                qkT.append(t_)

            v_sb = [[None] * 2 for _ in range(2)]
            for bb in range(2):
                for tt in range(2):
                    pv = ps.tile([128, C], FP32, tag="pb")
                    for ct in range(4):
                        nc.tensor.matmul(
                            pv[:],
                            xT2[ct][:, bb * T + tt * 128: bb * T + (tt + 1) * 128],
                            wqkv[:, ct, 2 * H * D:],
                            start=(ct == 0), stop=(ct == 3))
                    t_ = vp.tile([128, C], BF16, tag=f"v{bb}{tt}")
                    eng = nc.vector if (bb + tt) % 2 == 0 else nc.scalar
                    copy_to(eng, t_[:], pv[:])
                    v_sb[bb][tt] = t_

            # G batch tiles (one tile per jt-group of bands)
            gt = {}
            for bi, (t0, t1, jt) in enumerate(GBATCH):
                nb = t1 - t0
                g = gp.tile([128, nb, 2 * T], BF16, tag=f"g{bi}",
                            name=f"g{bi}_{pg}")
                gt[bi] = g
            nc.gpsimd.memset(gt[0][0:8, 0, :], 0.0)     # band0 rel0 pad
            # band18 tail pad: clear the whole band tile (full partition
            # range -- DVE partition windows are restricted); the gather
            # overwrites rows 0..40 afterwards
            nc.vector.memset(gt[3][:, 0, :], 0.0)

            E = ep.tile([128, H, 2, 2, T], BF16, tag="E", name=f"E_{pg}")

            def gather_batches(jt_done):
                # per-band single-dma gathers (the dma AP balancer caps
                # transfers at 3 refined dims, so bands can't be merged);
                # scalar is kept free of jt0-time issues (it is running exps)
                if jt_done == 0:
                    qs = (nc.sync, nc.gpsimd)
                    # band 0: rows -1..15 (rel 0 is the memset pad)
                    nc.sync.dma_start(gt[0][8:128, 0, :],
                                      E[0:15, :, 0, :, :])
                    for t in range(1, 9):
                        j0 = CONV_OUT * t
                        qs[t % 2].dma_start(gt[0][:, t, :],
                                            E[j0 - 1:j0 + 15, :, 0, :, :])
                    # band 9 jt0 part: rel 0..2 = rows 125..127
                    nc.gpsimd.dma_start(gt[1][0:24, 0, :],
                                        E[125:128, :, 0, :, :])
                else:
                    qs = (nc.sync, nc.scalar, nc.gpsimd)
                    # band 9 jt1 part: rel 3..15 = rows 128..140
                    nc.sync.dma_start(gt[1][24:128, 0, :],
                                      E[0:13, :, 1, :, :])
                    for t in range(10, 18):
                        j0 = CONV_OUT * t
                        qs[t % 3].dma_start(
                            gt[2][:, t - 10, :],
                            E[j0 - 129:j0 + 15 - 128, :, 1, :, :])
                    # band 18: rows 251..255 real, rest is the memset pad
                    nc.scalar.dma_start(gt[3][0:40, 0, :],
                                        E[123:128, :, 1, :, :])
            for jt in range(2):
                for h in range(H):
                    hp = 64 * (h % 2)
                    pd = ps.tile([128, 2 * T], FP32, tag="pb")
                    for bb in range(2):
                        nc.tensor.matmul(
                            pd[:, bb * T:(bb + 1) * T],
                            qkT[4 + h // 2][hp:hp + 64,
                                            bb * T + jt * 128: bb * T + (jt + 1) * 128],
                            qkT[h // 2][hp:hp + 64, bb * T:(bb + 1) * T],
                            start=True, stop=True)
                    nc.scalar.activation(E[:, h, jt, :, :], pd[:], EXP,
                                         scale=SCALE)
                gather_batches(jt)

            # softmax denominators
            sp16 = pss.tile([8, 2 * T], FP32, tag="s", name=f"sp_{pg}")
            for h in range(H):
                for jt in range(2):
                    nc.tensor.matmul(
                        sp16[:], ones8[:, h, :], E[:, h, jt, :, :],
                        start=(h == 0 and jt == 0),
                        stop=(h == H - 1 and jt == 1))
            rr16 = rp.tile([8, 2 * T], BF16, tag="rr")
            with nc.allow_low_precision(reason="bf16 recip"):
                nc.vector.reciprocal(rr16[:], sp16[:])

            # spectral conv (PE-filler while vector computes the recip)
            asc5 = {}
            for bb in range(2):
                a5 = s["asb5"][bb]
                c5 = ascp.tile([128, 5, C], BF16, tag=f"asc{bb}",
                               name=f"asc{bb}_{pg}")
                for k, (c0, m) in enumerate(SCHUNKS):
                    R = 128 if k < 4 else 10
                    ss = sstat0 if k == 0 else sstat
                    psc = ps.tile([128, C], FP32, tag="pb")
                    nc.tensor.matmul(psc[0:m, :], ss[0:R, 1, 0:m],
                                     a5[0:R, k, :], start=True, stop=False)
                    nc.tensor.matmul(psc[0:m, 1:C], ss[0:R, 0, 0:m],
                                     a5[0:R, k, 0:C - 1], start=False, stop=False)
                    nc.tensor.matmul(psc[0:m, 0:C - 1], ss[0:R, 2, 0:m],
                                     a5[0:R, k, 1:C], start=False, stop=True)
                    if (k + bb) % 2 == 0:
                        nc.scalar.activation(c5[0:m, k, :], psc[0:m, :], IDN,
                                             bias=bsp[0:m, 0:1])
                    else:
                        nc.vector.tensor_scalar(c5[0:m, k, :], psc[0:m, :],
                                                bsp[0:m, 0:1], None, ADD)
                asc5[bb] = c5

            Rg_ps = ps.tile([128, 2 * T], FP32, tag="pb", name="Rg_ps")
            nc.tensor.matmul(Rg_ps[:], sel16, rr16[:], start=True, stop=True)
            Rg = rp.tile([128, 2 * T], BF16, tag="Rg")
            nc.vector.tensor_copy(Rg[:], Rg_ps[:])
            Rg3 = Rg[:].rearrange("p (b i) -> p b i", b=2)

            # spatial conv
            AC = [acp.tile([128, H, 2, T], BF16, tag=f"AC{jt}",
                           name=f"AC{jt}_{pg}") for jt in range(2)]
            cgt = {bi: cgp.tile([112, t1 - t0, 2 * T], BF16, tag=f"cg{bi}",
                                name=f"cg{bi}_{pg}")
                   for bi, (t0, t1, jt) in enumerate(GBATCH)}

            for t in range(NBANDS):
                bi = next(i for i, (t0, t1, _) in enumerate(GBATCH)
                          if t0 <= t < t1)
                t0, t1, _ = GBATCH[bi]
                tb = t - t0
                g = gt[bi]
                nc.vector.tensor_tensor(g[:, tb, :], g[:, tb, :], Rg[:], MUL)
                nwid = 112 if t < 18 else 32
                pc = ps.tile([112, 2 * T], FP32, tag="pb")
                nc.tensor.matmul(pc[0:nwid, :], cstat[:, 1, 0:nwid],
                                 g[:, tb, :], start=True, stop=False)
                for bb in range(2):
                    nc.tensor.matmul(
                        pc[0:nwid, bb * T + 1:(bb + 1) * T],
                        cstat[:, 0, 0:nwid],
                        g[:, tb, bb * T:(bb + 1) * T - 1],
                        start=False, stop=False)
                    nc.tensor.matmul(
                        pc[0:nwid, bb * T:(bb + 1) * T - 1],
                        cstat[:, 2, 0:nwid],
                        g[:, tb, bb * T + 1:(bb + 1) * T],
                        start=False,
                        stop=(bb == 1 and not has_sconv_bias))
                if has_sconv_bias:
                    nc.tensor.matmul(pc[0:nwid, :], brow[:, 0:nwid],
                                     onesN, start=False, stop=True)
                ceng = (nc.vector, nc.scalar)[t % 2]
                copy_to(ceng, cgt[bi][0:nwid, tb, :], pc[0:nwid, :])
                # per-band scatter into AC (one dma; band 9 straddles jt)
                cg = cgt[bi]
                sq3 = (nc.sync, nc.gpsimd)[t % 2]
                j0 = CONV_OUT * t
                if t < 9:
                    sq3.dma_start(AC[0][j0:j0 + 14], cg[:, tb, :])
                elif t == 9:
                    nc.sync.dma_start(AC[0][126:128], cg[0:16, tb, :])
                    nc.gpsimd.dma_start(AC[1][0:12], cg[16:112, tb, :])
                elif t < 18:
                    sq3.dma_start(AC[1][j0 - 128:j0 - 128 + 14],
                                  cg[:, tb, :])
                else:
                    sq3.dma_start(AC[1][124:128], cg[0:32, tb, :])

            s.update(qkT=qkT, v_sb=v_sb, AC=AC, asc5=asc5)

        # ================= tail: attn@v, proj, final, store ==============
        def emit_tail(pg):
            s = st[pg]
            v_sb, AC, asc5 = s["v_sb"], s["AC"], s["asc5"]

            OF = []
            for g4 in range(4):
                pav = ps.tile([128, 2 * T], FP32, tag="pb", name="pav")
                for hh in range(2):
                    h = 2 * g4 + hh
                    for bb in range(2):
                        for jt in range(2):
                            nc.tensor.matmul(
                                pav[64 * hh:64 * hh + 64, bb * T:(bb + 1) * T],
                                v_sb[bb][jt][:, h * D:(h + 1) * D],
                                AC[jt][:, h, bb, :],
                                start=(jt == 0), stop=(jt == 1),
                                tile_position=(0, 64 * hh))
                t_ = ofp.tile([128, 2 * T], BF16, tag=f"of{g4}")
                eng = nc.vector if g4 % 2 == 0 else nc.scalar
                copy_to(eng, t_[:], pav[:])
                OF.append(t_)

            outT5 = []
            for k, (c0, m) in enumerate(SCHUNKS):
                pp = ps.tile([128, 2 * T], FP32, tag="pb", name="pp")
                for kt in range(4):
                    nc.tensor.matmul(
                        pp[0:m, :], wout[:, kt, c0:c0 + m], OF[kt][:],
                        start=(kt == 0), stop=(kt == 3))
                t_ = otp.tile([128, 2 * T], BF16, tag=f"ot{k}")
                if k % 2 == 0:
                    nc.vector.tensor_scalar(t_[0:m, :], pp[0:m, :],
                                            b_out5[0:m, k:k + 1], None, ADD)
                else:
                    nc.scalar.activation(t_[0:m, :], pp[0:m, :], IDN,
                                         bias=b_out5[0:m, k:k + 1])
                outT5.append(t_)

            for bb in range(2):
                res = resp.tile([128, 2, C], BF16, tag=f"res{bb}")
                for tt2 in range(2):
                    pf = ps.tile([128, C], FP32, tag="pb", name="pf")
                    for k, (c0, m) in enumerate(SCHUNKS):
                        nc.tensor.matmul(
                            pf[:],
                            outT5[k][0:m, bb * T + tt2 * 128: bb * T + (tt2 + 1) * 128],
                            asc5[bb][0:m, k, :],
                            start=(k == 0), stop=(k == 4))
                    ceng = (nc.vector, nc.scalar)[(2 * bb + tt2) % 2]
                    copy_to(ceng, res[:, tt2, :], pf[:])
                (nc.sync if bb == 0 else nc.scalar).dma_start(
                    out_d[2 * pg + bb].rearrange("(tt p) c -> p tt c", p=128),
                    res[:])

        emit_head(0)
        emit_mid(0)
        emit_head(1)
        emit_tail(0)
        emit_mid(1)
        emit_tail(1)

    nc.compile()
    return nc


def _prep_inputs(inputs):
    x = np.asarray(inputs["x"], np.float32)
    w_qkv = np.asarray(inputs["w_qkv"], np.float32)
    w_out = np.asarray(inputs["w_out"], np.float32)
    b_out = np.asarray(inputs["b_out"], np.float32)
    w_sconv = np.asarray(inputs["w_sconv"], np.float32)
    b_sconv = np.asarray(inputs["b_sconv"], np.float32)
    w_specconv = np.asarray(inputs["w_specconv"], np.float32)
    b_specconv = np.asarray(inputs["b_specconv"], np.float32)
    w_qkv_spec = np.asarray(inputs["w_qkv_spec"], np.float32)

    has_sconv_bias = bool(np.any(b_sconv != 0))

    OFF, TOT = _blob_layout()
    blob = np.zeros((128, TOT), np.float32)

    def put(name, arr):
        a, b = OFF[name]
        blob[:arr.shape[0], a:b] = arr.reshape(arr.shape[0], b - a)

    put("ident", np.eye(128, dtype=np.float32))
    wspq = np.stack([w_qkv_spec[tt * 128:(tt + 1) * 128, :T]
                     for tt in range(2)], axis=1)
    put("wspq", wspq)
    wspk = np.stack([w_qkv_spec[tt * 128:(tt + 1) * 128, T:2 * T]
                     for tt in range(2)], axis=1)
    put("wspk", wspk)
    put("cstat", build_conv_stationaries(w_sconv).transpose(1, 0, 2))
    put("sstat", build_spec_stationaries(w_specconv[0, 0]))
    put("sstat0", build_spec_stationaries(w_specconv[0, 0], shift=1))
    put("ones8", _ones8())
    put("sel16", _sel16())
    put("brow", np.tile(b_sconv, CONV_OUT)[None, :])
    put("onesN", np.ones((1, C), np.float32))
    put("wout", np.stack([w_out[kt * 128:(kt + 1) * 128]
                          for kt in range(4)], axis=1))
    put("wqkv", np.stack([w_qkv[ct * 128:(ct + 1) * 128]
                          for ct in range(4)], axis=1))

    fblob = np.zeros((128, 6), np.float32)
    for k, (c0, m) in enumerate(SCHUNKS):
        fblob[0:m, k] = b_out[c0:c0 + m]
    fblob[:, 5] = b_specconv[0]

    common = {"wblob": _to_bf16(blob), "fblob": fblob}
    in_maps = []
    for core in range(NCORES):
        m = dict(common)
        m["x"] = _to_bf16(x[core * NB:(core + 1) * NB])
        in_maps.append(m)
    return in_maps, has_sconv_bias


def kernel(**inputs):
    in_maps, has_sconv_bias = _prep_inputs(inputs)
    key = ("v2", has_sconv_bias)
    if key not in _cache:
        _cache[key] = build_program(has_sconv_bias)
    nc = _cache[key]
    trace = bool(int(os.environ.get("KERNEL_TRACE", "0")))
    res = run_bass_kernel_spmd(nc, in_maps, list(range(NCORES)), trace=trace)
    if trace and res.exec_time_ns is not None:
        kernel.last_exec_time_ns = res.exec_time_ns
        kernel.last_profile = res
    out = np.concatenate(
        [np.asarray(res.results[i]["out"]).astype(np.float32)
         for i in range(NCORES)], axis=0)
    return out


kernel.last_exec_time_ns = None
kernel.last_profile = None
